# revision 5
# baseline (speedup 1.0000x reference)
"""HGCN (2x hyperbolic GCN layer + MLP head) as a distributed Bass/Tile kernel
for 8 trn2 NeuronCores — v2: SWDGE dma_gather edge gather.

Math (as baseline): logmap0(expmap0(v)) == v for this data, so
    t2  = sigmoid(meanagg(X) @ W1 + b1)
    t3  = sigmoid(meanagg(t2) @ W2 + b2)
    out = relu(t3 @ W3 + b3) @ W4 + b4

Distribution: destination nodes sharded 8 ways (12500/core + 44 pad = 12544
= T*128 rows, T=98 tiles).  Sources are gathered from a replicated f32 table
(25.6MB; layer 2's table arrives via AllGather).  dma_gather indices are
int16, so the table is split into 4 windows of 25088 rows; shard 2w's zero
pad rows sit at relative row 12500 inside window w, giving every window a
zero PAD target.

Slot grid per core: for window w, tile t gets nw[t][w] slot columns; tiles
are grouped into chunks with a uniform per-tile column count (padded to the
chunk max), so one dma_gather fills [128, ntile*nbar, 64] and the tree
reduce runs as log2(nbar) big strided DVE adds, accumulated per tile into a
persistent ACC [128, T, 64].  Nodes are packed into tiles by a greedy vector
bin-packing that minimizes the per-window column maxima.
"""

import os
import numpy as np

import concourse.bass as bass
import concourse.bacc as bacc
import concourse.tile as tile
from concourse import mybir
from concourse.masks import make_identity

NC = 8
P = 128
D = 64
NW = 4
CHUNK_COLS = 64    # max slot columns per gather chunk; 64*128 = 8192 idxs
                   # (multi-packet SWDGE gather caps at 8192 descriptors)

BF16 = mybir.dt.bfloat16
F32 = mybir.dt.float32
I16 = mybir.dt.int16

N_NODES = 100000
SH = N_NODES // NC            # 12500
T = (SH + P - 1) // P         # 98
SHP = T * P                   # 12544
NROWS = NC * SHP              # 100352
WSZ = NROWS // NW             # 25088
PADROW = SH                   # relative zero row inside every window


def _pack_tiles(cntk):
    """Order shard nodes (rows of cntk [n, NW]) into tiles of 128: greedy
    vector packing inside 16-tile blocks (minimizes per-tile window maxima),
    tiles within a block reordered by decreasing max so adaptive chunks pad
    little."""
    n = len(cntk)
    osort = np.argsort(-(cntk.max(1) * 1000 + cntk.sum(1)), kind="stable")
    B = 16 * P
    out = []
    for s0 in range(0, n, B):
        blk = osort[s0:s0 + B]
        ntile = (len(blk) + P - 1) // P
        maxs = np.zeros((ntile, NW), np.int64)
        fills = np.zeros(ntile, np.int64)
        asn = [[] for _ in range(ntile)]
        for nd in blk:
            cv = cntk[nd]
            inc = (np.maximum(maxs, cv) - maxs).sum(1)
            inc[fills >= P] = 1 << 30
            ti = int(np.argmin(inc))
            asn[ti].append(nd)
            fills[ti] += 1
            np.maximum(maxs[ti], cv, out=maxs[ti])
        key = [-(maxs[ti].max() * 1000 + maxs[ti].sum()) for ti in range(ntile)]
        for ti in np.argsort(key, kind="stable"):
            out.extend(asn[ti])
    return np.asarray(out, np.int64)


def _preprocess(edge_index):
    """Host-side layout preprocessing (index shuffling only)."""
    src = np.asarray(edge_index[0], np.int64)
    dst = np.asarray(edge_index[1], np.int64)
    deg = np.bincount(dst, minlength=N_NODES).astype(np.int64)
    deg_out = np.bincount(src, minlength=N_NODES).astype(np.int64)

    # --- window assignment: choose each node's shard-PAIR (= its gather
    # window when it appears as a source) to balance every destination's
    # in-neighborhood across the 4 windows.
    order = np.argsort(src, kind="stable")
    dst_by_src = dst[order]
    ptr = np.zeros(N_NODES + 1, np.int64)
    np.cumsum(np.bincount(src, minlength=N_NODES), out=ptr[1:])
    targ = np.ceil(deg / NW).astype(np.int64)
    cnt = np.zeros((N_NODES, NW), np.int32)     # per (dst, window) counts
    cap = 2 * SH                                # 25000 nodes per pair
    sizes = np.zeros(NW, np.int64)
    assign = np.full(N_NODES, -1, np.int64)
    for s in np.argsort(-deg_out, kind="stable"):
        nbrs = dst_by_src[ptr[s]:ptr[s + 1]]
        if len(nbrs):
            over = (cnt[nbrs] >= targ[nbrs][:, None]).sum(0)
        else:
            over = np.zeros(NW, np.int64)
        g = int(np.argmin(over + (sizes >= cap) * 10 ** 9
                          + 0.001 * (sizes / cap)))
        assign[s] = g
        sizes[g] += 1
        if len(nbrs):
            cnt[nbrs, g] += 1
    # the += above undercounts duplicate edges; recompute exact counts
    cnt = np.zeros((N_NODES, NW), np.int64)
    np.add.at(cnt, (dst, assign[src]), 1)

    # pair -> two shards, alternating by in-degree for load balance
    shard_of = np.empty(N_NODES, np.int8)
    for g in range(NW):
        nodes = np.flatnonzero(assign == g)
        so = nodes[np.argsort(-deg[nodes], kind="stable")]
        shard_of[so[0::2]] = 2 * g
        shard_of[so[1::2]] = 2 * g + 1

    # per-shard tile packing
    perm = np.empty((NC, SH), np.int64)
    row_of = np.empty(N_NODES, np.int64)
    for k in range(NC):
        nodes = np.flatnonzero(shard_of == k)
        perm[k] = nodes[_pack_tiles(cnt[nodes])]
        row_of[perm[k]] = k * SHP + np.arange(SH)
    w_fin = assign[src]                          # == row_of[src] // WSZ

    # per-(tile,window) column counts, uniform across cores
    cc = np.zeros((NC, T, P, NW), np.int64)
    for k in range(NC):
        cv = np.vstack([cnt[perm[k]], np.zeros((SHP - SH, NW), np.int64)])
        cc[k] = cv.reshape(T, P, NW)
    nw = cc.max(axis=(0, 2))                       # [T, NW] per-tile need
    nw[:, 0] = np.maximum(nw[:, 0], 1)             # window 0 covers all tiles

    # chunks per window: consecutive tiles, per-tile count padded to chunk
    # max (nbar); one dma_gather per chunk.
    wchunks = [[] for _ in range(NW)]              # (t0, t1, nbar)
    for w in range(NW):
        t0 = 0
        while t0 < T:
            t1 = t0
            nbar = 0
            while t1 < T:
                nb = max(nbar, int(nw[t1, w]))
                if (t1 - t0 + 1) * nb > CHUNK_COLS and t1 > t0:
                    break
                nbar = nb
                t1 += 1
            if nbar == 0:
                t1 = min(t0 + CHUNK_COLS, T)       # all-zero stretch
                wchunks[w].append((t0, t1, 0))
            else:
                wchunks[w].append((t0, t1, nbar))
            t0 = t1
    # realized per-tile slot width
    nbar_tw = np.zeros((T, NW), np.int64)
    for w in range(NW):
        for (t0, t1, nbar) in wchunks[w]:
            nbar_tw[t0:t1, w] = nbar
    C = int(nbar_tw.sum())

    # slot values: for each core, per (t, w): list of window-relative source
    # rows of window-w in-edges of each node, padded to nbar_tw[t,w]
    r = row_of[dst]
    k_e = r // SHP
    q = r % SHP
    t_e = q // P
    p_e = q % P
    okey = (((k_e * T + t_e) * P + p_e) * NW + w_fin)
    order = np.argsort(okey, kind="stable")
    ks, ts, ps, ws = k_e[order], t_e[order], p_e[order], w_fin[order]
    val = (row_of[src] - w_fin * WSZ)[order]
    ok_s = okey[order]
    first = np.r_[True, ok_s[1:] != ok_s[:-1]]
    starts = np.flatnonzero(first)
    gid = np.cumsum(first) - 1
    j = np.arange(len(ok_s)) - starts[gid]

    # slot array organized [NC, T, NW, nbar, P] ragged via flat offsets
    slotbase = np.concatenate([[0], np.cumsum(nbar_tw.reshape(-1))])  # T*NW+1
    slots = np.full((NC, int(slotbase[-1]), P), PADROW, np.int32)
    sidx = slotbase[ts * NW + ws] + j
    slots[ks, sidx, ps] = val

    # int16 packed idx stream per (w, chunk): flat i = (trel*nbar + c)*128+p
    # -> [p%16, i//16]; replicated over the 16-row groups of 128 partitions.
    # order segments by end tile so early tiles' accumulators complete (and
    # their per-tile chains can overlap) while later segments still gather
    raw = []
    for w in range(NW):
        for (t0, t1, nbar) in wchunks[w]:
            if nbar == 0:
                continue
            raw.append((w, t0, t1, nbar))
    raw.sort(key=lambda s: (s[2], s[1], s[0]))
    seglist = []       # (w, t0, t1, nbar, ioff, n_idx)
    ioff = 0
    for (w, t0, t1, nbar) in raw:
        n_idx = (t1 - t0) * nbar * P
        seglist.append((w, t0, t1, nbar, ioff, n_idx))
        ioff += n_idx // 16
    IDXC = ioff

    idxpack = np.empty((NC, P, IDXC), np.int16)
    for k in range(NC):
        for (w, t0, t1, nbar, off, n_idx) in seglist:
            vs = [slots[k, slotbase[t * NW + w]:slotbase[t * NW + w] + nbar]
                  for t in range(t0, t1)]
            m = np.concatenate(vs, 0).reshape(n_idx)      # (trel, c, p) flat
            blk = m.reshape(n_idx // 16, 16).T            # [16, n_idx/16]
            idxpack[k, :, off:off + n_idx // 16] = np.tile(blk, (8, 1))

    # per-(tile,partition) 1/max(deg,1)
    dinv = np.zeros((NC, P, T), np.float32)
    for k in range(NC):
        dv = (1.0 / np.maximum(deg[perm[k]], 1)).astype(np.float32)
        dv = np.pad(dv, (0, SHP - SH))
        dinv[k] = dv.reshape(T, P).T

    return dict(C=C, seglist=seglist, idxpack=idxpack, dinv=dinv, perm=perm,
                IDXC=IDXC, nbar_tw=nbar_tw)


def _build_program(meta):
    seglist, IDXC = meta["seglist"], meta["IDXC"]

    nc = bacc.Bacc("TRN2", target_bir_lowering=False, debug=False,
                   enable_asserts=False, num_devices=NC, num_swdge_queues=4)

    xtab_d = nc.dram_tensor("xtab", [NROWS, D], F32, kind="ExternalInput")
    idx_d = nc.dram_tensor("idx", [P, IDXC], I16, kind="ExternalInput")
    dinv_d = nc.dram_tensor("dinv", [P, T], F32, kind="ExternalInput")
    pmask_d = nc.dram_tensor("pmask", [P, 1], F32, kind="ExternalInput")
    w1_d = nc.dram_tensor("w1", [D, D], BF16, kind="ExternalInput")
    w2_d = nc.dram_tensor("w2", [D, D], BF16, kind="ExternalInput")
    w3_d = nc.dram_tensor("w3", [D, P], BF16, kind="ExternalInput")
    w4_d = nc.dram_tensor("w4", [P, 40], BF16, kind="ExternalInput")
    b1_d = nc.dram_tensor("b1", [D, 1], F32, kind="ExternalInput")
    b2_d = nc.dram_tensor("b2", [D, 1], F32, kind="ExternalInput")
    b3_d = nc.dram_tensor("b3", [P, 1], F32, kind="ExternalInput")
    b4_d = nc.dram_tensor("b4", [40, 1], F32, kind="ExternalInput")
    outT_d = nc.dram_tensor("outT", [40, SHP], F32, kind="ExternalOutput")

    t2self = nc.dram_tensor("t2self", [SHP, D], F32)
    t2full = nc.dram_tensor("t2full", [NROWS, D], F32)

    from contextlib import ExitStack
    with tile.TileContext(nc) as tc, ExitStack() as es:
        const = es.enter_context(tc.tile_pool(name="const", bufs=1))
        gpool = es.enter_context(tc.tile_pool(name="gpool", bufs=2))
        small = es.enter_context(tc.tile_pool(name="small", bufs=3))
        psum = es.enter_context(tc.tile_pool(name="psum", bufs=3, space="PSUM"))

        idx_s = const.tile([P, IDXC], I16)
        ic0 = seglist[0][5] // 16 if seglist else IDXC
        nc.sync.dma_start(out=idx_s[:, :ic0], in_=idx_d[:, :ic0])
        nc.sync.dma_start(out=idx_s[:, ic0:], in_=idx_d[:, ic0:])
        dinv_s = const.tile([P, T], F32)
        nc.sync.dma_start(out=dinv_s[:], in_=dinv_d[:])
        pmask_s = const.tile([P, 1], F32)
        nc.sync.dma_start(out=pmask_s[:], in_=pmask_d[:])
        w1_s = const.tile([D, D], BF16)
        nc.sync.dma_start(out=w1_s[:], in_=w1_d[:])
        w2_s = const.tile([D, D], BF16)
        nc.sync.dma_start(out=w2_s[:], in_=w2_d[:])
        w3_s = const.tile([D, P], BF16)
        nc.sync.dma_start(out=w3_s[:], in_=w3_d[:])
        w4_s = const.tile([P, 40], BF16)
        nc.sync.dma_start(out=w4_s[:], in_=w4_d[:])
        b1_s = const.tile([D, 1], F32)
        nc.sync.dma_start(out=b1_s[:], in_=b1_d[:])
        b2_s = const.tile([D, 1], F32)
        nc.sync.dma_start(out=b2_s[:], in_=b2_d[:])
        b3_s = const.tile([P, 1], F32)
        nc.sync.dma_start(out=b3_s[:], in_=b3_d[:])
        b4_s = const.tile([40, 1], F32)
        nc.sync.dma_start(out=b4_s[:], in_=b4_d[:])
        ident = const.tile([P, P], F32)
        make_identity(nc, ident[:])
        outT_s = const.tile([40, SHP], F32)
        acc = const.tile([P, T, D], BF16)

        # tiles become final after their last covering segment (in seglist
        # order); emit their per-tile chains right there so chain work
        # overlaps later segments' gathers/trees on every engine stream
        last_seg = [0] * T
        for si, (w, t0, t1, nbar, off, n_idx) in enumerate(seglist):
            for t in range(t0, t1):
                last_seg[t] = max(last_seg[t], si)
        done_after = [[] for _ in seglist]
        for t in range(T):
            done_after[last_seg[t]].append(t)

        def layer(tab_ap, w_s, b_s, last):
            def chain(t, aggs_g, a):
                pt = psum.tile([D, P], F32, tag="tp", space="PSUM")
                nc.tensor.transpose(pt[:], aggs_g[:, t - a, :], ident[:])
                rhs = small.tile([D, P], BF16, tag="rhs")
                nc.scalar.activation(
                    rhs[:], pt[:], mybir.ActivationFunctionType.Copy)
                pm = psum.tile([D, P], F32, tag="mm", space="PSUM")
                nc.tensor.matmul(pm[:], lhsT=w_s[:], rhs=rhs[:],
                                 start=True, stop=True)
                tT = small.tile([D, P], BF16 if last else F32, tag="tT")
                nc.scalar.activation(
                    tT[:], pm[:], mybir.ActivationFunctionType.Sigmoid,
                    bias=b_s[:, :1])
                if not last:
                    pb = psum.tile([P, D], F32, tag="tp", space="PSUM")
                    nc.tensor.transpose(pb[:], tT[:], ident[:D, :D])
                    t2t = small.tile([P, D], F32, tag="t2t")
                    # Act copy; the T-1 tile zeroes the 44 shard-pad rows via
                    # the pmask scale so every window keeps a zero PAD target
                    nc.scalar.activation(
                        t2t[:], pb[:], mybir.ActivationFunctionType.Copy,
                        scale=pmask_s[:, :1] if t == T - 1 else 1.0)
                    nc.sync.dma_start(
                        out=t2self[t * P:(t + 1) * P, :], in_=t2t[:])
                else:
                    p3 = psum.tile([P, P], F32, tag="mm", space="PSUM")
                    nc.tensor.matmul(p3[:], lhsT=w3_s[:], rhs=tT[:],
                                     start=True, stop=True)
                    h3 = small.tile([P, P], BF16, tag="h3")
                    nc.scalar.activation(
                        h3[:], p3[:], mybir.ActivationFunctionType.Relu,
                        bias=b3_s[:, :1])
                    p4 = psum.tile([40, P], F32, tag="mm", space="PSUM")
                    nc.tensor.matmul(p4[:], lhsT=w4_s[:], rhs=h3[:],
                                     start=True, stop=True)
                    nc.scalar.activation(
                        outT_s[:, t * P:(t + 1) * P], p4[:],
                        mybir.ActivationFunctionType.Identity,
                        bias=b4_s[:, :1])

            nc.vector.memset(acc[:], 0.0)
            for si, (w, t0, t1, nbar, off, n_idx) in enumerate(seglist):
                ntile = t1 - t0
                G = gpool.tile([P, ntile, nbar, D], F32, tag="G")
                Gfull = G[:]
                G3 = bass.AP(tensor=Gfull.tensor, offset=Gfull.offset,
                             ap=[list(Gfull.ap[0]), [D, ntile * nbar], [1, D]])
                assert n_idx <= 8192, (
                    f"gather of {n_idx} idxs exceeds the 8192-descriptor "
                    f"SWDGE limit (crashes the device)")
                nc.gpsimd.dma_gather(
                    out_ap=G3,
                    in_ap=tab_ap[w * WSZ:(w + 1) * WSZ],
                    idxs_ap=idx_s[:, off:off + n_idx // 16],
                    num_idxs=n_idx, num_idxs_reg=n_idx,
                    elem_size=D, queue_num=si % 4, single_packet=False)
                # tree-reduce axis c: level 0 folds f32 pairs into a
                # contiguous bf16 tile (emulated precision cost: +4e-4 rel),
                # later levels run in bf16 at 2x DVE rate
                if nbar == 1:
                    radd = Gfull[:, :, 0, :]
                else:
                    n = nbar
                    if n % 2:
                        nc.vector.tensor_tensor(
                            out=Gfull[:, :, 0, :], in0=Gfull[:, :, 0, :],
                            in1=Gfull[:, :, n - 1, :], op=mybir.AluOpType.add)
                    h = n // 2
                    Gb = gpool.tile([P, ntile, h, D], BF16, tag="Gb")
                    nc.vector.tensor_tensor(
                        out=Gb[:], in0=Gfull[:, :, :h, :],
                        in1=Gfull[:, :, h:2 * h, :], op=mybir.AluOpType.add)
                    n = h
                    while n > 1:
                        h = n // 2
                        if n % 2:
                            nc.vector.tensor_tensor(
                                out=Gb[:, :, 0, :], in0=Gb[:, :, 0, :],
                                in1=Gb[:, :, n - 1, :], op=mybir.AluOpType.add)
                        nc.vector.tensor_tensor(
                            out=Gb[:, :, :h, :], in0=Gb[:, :, :h, :],
                            in1=Gb[:, :, h:2 * h, :], op=mybir.AluOpType.add)
                        n = h
                    radd = Gb[:, :, 0, :]
                nc.vector.tensor_tensor(
                    out=acc[:, t0:t1, :], in0=acc[:, t0:t1, :],
                    in1=radd, op=mybir.AluOpType.add)
                grp = done_after[si]
                if grp:
                    a, b = grp[0], grp[-1] + 1
                    aggs_g = small.tile([P, b - a, D], F32, tag="aggs")
                    dv = dinv_s[:, a:b]
                    dvb = bass.AP(tensor=dv.tensor, offset=dv.offset,
                                  ap=[list(dv.ap[0]), list(dv.ap[1]), [0, D]])
                    nc.vector.tensor_tensor(
                        out=aggs_g[:], in0=acc[:, a:b, :], in1=dvb,
                        op=mybir.AluOpType.mult)
                    for t in grp:
                        chain(t, aggs_g, a)

        layer(xtab_d[:], w1_s, b1_s, last=False)
        nc.gpsimd.collective_compute(
            "AllGather",
            mybir.AluOpType.bypass,
            replica_groups=[list(range(NC))],
            ins=[t2self.ap().opt()],
            outs=[t2full.ap().opt()],
        )
        layer(t2full[:], w2_s, b2_s, last=True)
        nc.sync.dma_start(out=outT_d[:], in_=outT_s[:])

    nc.compile()
    return nc


def kernel(features, edge_index, W1, b1, W2, b2, W3, b3, W4, b4):
    meta = _preprocess(edge_index)
    perm = meta["perm"]

    nc = _build_program(meta)

    xtab = np.zeros((NROWS, D), np.float32)
    X = np.asarray(features, np.float32)
    for k in range(NC):
        xtab[k * SHP:k * SHP + SH] = X[perm[k]]

    pmask = np.zeros((P, 1), np.float32)
    pmask[:SH - (T - 1) * P] = 1.0
    common = dict(
        xtab=xtab,
        pmask=pmask,
        w1=np.asarray(W1, np.float32).astype(np.float32),
        w2=np.asarray(W2, np.float32).astype(np.float32),
        w3=np.asarray(W3, np.float32).astype(np.float32),
        w4=np.asarray(W4, np.float32).astype(np.float32),
        b1=np.asarray(b1, np.float32).reshape(D, 1),
        b2=np.asarray(b2, np.float32).reshape(D, 1),
        b3=np.asarray(b3, np.float32).reshape(P, 1),
        b4=np.asarray(b4, np.float32).reshape(40, 1),
    )
    # weights are declared bf16 on device; ml_dtypes cast here
    import ml_dtypes
    for wn in ("w1", "w2", "w3", "w4"):
        common[wn] = common[wn].astype(ml_dtypes.bfloat16)

    in_maps = [dict(common, idx=meta["idxpack"][k],
                    dinv=meta["dinv"][k]) for k in range(NC)]

    results = _run_spmd_timed(nc, in_maps,
                              reps=int(os.environ.get("KERNEL_REPS", "0")))

    out = np.empty((N_NODES, 40), np.float32)
    for k in range(NC):
        outT = np.asarray(results[k]["outT"], np.float32)
        out[perm[k]] = outT[:, :SH].T
    return out


def _run_spmd_timed(nc, in_maps, reps=0):
    """Device_put inputs once; repeated warm timed executions (NTFF profiling
    is unavailable under this axon client, so warm wall-clock is the
    metric)."""
    import time
    import jax
    from jax.sharding import Mesh, PartitionSpec
    from jax.experimental.shard_map import shard_map
    from concourse import bass2jax, mybir as mb

    bass2jax.install_neuronx_cc_hook()
    n_cores = len(in_maps)
    partition_name = (nc.partition_id_tensor.name
                      if nc.partition_id_tensor else None)
    in_names, out_names, out_avals, zero_outs = [], [], [], []
    for alloc in nc.m.functions[0].allocations:
        if not isinstance(alloc, mb.MemoryLocationSet):
            continue
        name = alloc.memorylocations[0].name
        if alloc.kind == "ExternalInput":
            if name != partition_name:
                in_names.append(name)
        elif alloc.kind == "ExternalOutput":
            shape = tuple(alloc.tensor_shape)
            dtype = mb.dt.np(alloc.dtype)
            out_avals.append(jax.core.ShapedArray(shape, dtype))
            zero_outs.append(np.zeros(shape, dtype))
            out_names.append(name)
    n_params = len(in_names)
    n_outs = len(out_avals)
    all_in_names = list(in_names) + list(out_names)
    if partition_name is not None:
        all_in_names.append(partition_name)

    def _body(*args):
        operands = list(args)
        if partition_name is not None:
            operands.append(bass2jax.partition_id_tensor())
        return tuple(bass2jax._bass_exec_p.bind(
            *operands, out_avals=tuple(out_avals),
            in_names=tuple(all_in_names), out_names=tuple(out_names),
            lowering_input_output_aliases=(),
            sim_require_finite=True, sim_require_nnan=True, nc=nc))

    devices = jax.devices()[:n_cores]
    mesh = Mesh(np.asarray(devices), ("core",))
    sharded = jax.jit(
        shard_map(_body, mesh=mesh,
                  in_specs=(PartitionSpec("core"),) * (n_params + n_outs),
                  out_specs=(PartitionSpec("core"),) * n_outs,
                  check_rep=False),
        keep_unused=True)

    concat_in = [np.concatenate([np.asarray(m[name]) for m in in_maps], axis=0)
                 for name in in_names]
    dev_in = [jax.device_put(a) for a in concat_in]
    jax.block_until_ready(dev_in)

    dev_zeros = [jax.device_put(np.zeros((n_cores * z.shape[0],
                                          *z.shape[1:]), z.dtype))
                 for z in zero_outs]
    jax.block_until_ready(dev_zeros)

    def one_call():
        t0 = time.perf_counter()
        outs = sharded(*dev_in, *dev_zeros)
        jax.block_until_ready(outs)
        return time.perf_counter() - t0, outs

    _, outs = one_call()            # compile + first exec
    if reps > 0:
        times = [one_call()[0] for _ in range(reps)]
        best = min(times)
        print(f"HW exec time: {best * 1e9:.0f} ns")
        print("wall times (s):", [f"{t:.4f}" for t in times])
    return [
        {name: np.asarray(outs[i]).reshape(n_cores, *out_avals[i].shape)[c]
         for i, name in enumerate(out_names)}
        for c in range(n_cores)
    ]


if __name__ == "__main__":
    d = np.load("/tmp/inputs.npz")
    out = kernel(**{k: d[k] for k in d.files})
    ref = np.load("/tmp/ref.npy")
    err = np.abs(out - ref).max() / np.abs(ref).max()
    print("Relative error:", err)


# revision 6
# speedup vs baseline: 1.0362x; 1.0362x over previous
"""HGCN (2x hyperbolic GCN layer + MLP head) as a distributed Bass/Tile kernel
for 8 trn2 NeuronCores — v2: SWDGE dma_gather edge gather.

Math (as baseline): logmap0(expmap0(v)) == v for this data, so
    t2  = sigmoid(meanagg(X) @ W1 + b1)
    t3  = sigmoid(meanagg(t2) @ W2 + b2)
    out = relu(t3 @ W3 + b3) @ W4 + b4

Distribution: destination nodes sharded 8 ways (12500/core + 44 pad = 12544
= T*128 rows, T=98 tiles).  Sources are gathered from a replicated f32 table
(25.6MB; layer 2's table arrives via AllGather).  dma_gather indices are
int16, so the table is split into 4 windows of 25088 rows; shard 2w's zero
pad rows sit at relative row 12500 inside window w, giving every window a
zero PAD target.

Slot grid per core: for window w, tile t gets nw[t][w] slot columns; tiles
are grouped into chunks with a uniform per-tile column count (padded to the
chunk max), so one dma_gather fills [128, ntile*nbar, 64] and the tree
reduce runs as log2(nbar) big strided DVE adds, accumulated per tile into a
persistent ACC [128, T, 64].  Nodes are packed into tiles by a greedy vector
bin-packing that minimizes the per-window column maxima.
"""

import os
import numpy as np

import concourse.bass as bass
import concourse.bacc as bacc
import concourse.tile as tile
from concourse import mybir
from concourse.masks import make_identity

NC = 8
P = 128
D = 64
NW = 4
CHUNK_COLS = 28    # max slot columns per gather chunk (28*128 = 3584 idxs,
                   # well under the 8192-descriptor multi-packet SWDGE cap;
                   # smaller chunks pad less, and Pool has dispatch slack)

BF16 = mybir.dt.bfloat16
F32 = mybir.dt.float32
I16 = mybir.dt.int16

N_NODES = 100000
SH = N_NODES // NC            # 12500
T = (SH + P - 1) // P         # 98
SHP = T * P                   # 12544
NROWS = NC * SHP              # 100352
WSZ = NROWS // NW             # 25088
PADROW = SH                   # relative zero row inside every window


def _pack_tiles(cntk):
    """Order shard nodes (rows of cntk [n, NW]) into tiles of 128: greedy
    vector packing inside 16-tile blocks (minimizes per-tile window maxima),
    tiles within a block reordered by decreasing max so adaptive chunks pad
    little."""
    n = len(cntk)
    osort = np.argsort(-(cntk.max(1) * 1000 + cntk.sum(1)), kind="stable")
    B = 16 * P
    out = []
    for s0 in range(0, n, B):
        blk = osort[s0:s0 + B]
        ntile = (len(blk) + P - 1) // P
        maxs = np.zeros((ntile, NW), np.int64)
        fills = np.zeros(ntile, np.int64)
        asn = [[] for _ in range(ntile)]
        for nd in blk:
            cv = cntk[nd]
            inc = (np.maximum(maxs, cv) - maxs).sum(1)
            inc[fills >= P] = 1 << 30
            ti = int(np.argmin(inc))
            asn[ti].append(nd)
            fills[ti] += 1
            np.maximum(maxs[ti], cv, out=maxs[ti])
        key = [-(maxs[ti].max() * 1000 + maxs[ti].sum()) for ti in range(ntile)]
        for ti in np.argsort(key, kind="stable"):
            out.extend(asn[ti])
    return np.asarray(out, np.int64)


def _preprocess(edge_index):
    """Host-side layout preprocessing (index shuffling only)."""
    src = np.asarray(edge_index[0], np.int64)
    dst = np.asarray(edge_index[1], np.int64)
    deg = np.bincount(dst, minlength=N_NODES).astype(np.int64)
    deg_out = np.bincount(src, minlength=N_NODES).astype(np.int64)

    # --- window assignment: choose each node's shard-PAIR (= its gather
    # window when it appears as a source) to balance every destination's
    # in-neighborhood across the 4 windows.
    order = np.argsort(src, kind="stable")
    dst_by_src = dst[order]
    ptr = np.zeros(N_NODES + 1, np.int64)
    np.cumsum(np.bincount(src, minlength=N_NODES), out=ptr[1:])
    targ = np.ceil(deg / NW).astype(np.int64)
    cnt = np.zeros((N_NODES, NW), np.int32)     # per (dst, window) counts
    cap = 2 * SH                                # 25000 nodes per pair
    sizes = np.zeros(NW, np.int64)
    assign = np.full(N_NODES, -1, np.int64)
    for s in np.argsort(-deg_out, kind="stable"):
        nbrs = dst_by_src[ptr[s]:ptr[s + 1]]
        if len(nbrs):
            over = (cnt[nbrs] >= targ[nbrs][:, None]).sum(0)
        else:
            over = np.zeros(NW, np.int64)
        g = int(np.argmin(over + (sizes >= cap) * 10 ** 9
                          + 0.001 * (sizes / cap)))
        assign[s] = g
        sizes[g] += 1
        if len(nbrs):
            cnt[nbrs, g] += 1
    # the += above undercounts duplicate edges; recompute exact counts
    cnt = np.zeros((N_NODES, NW), np.int64)
    np.add.at(cnt, (dst, assign[src]), 1)

    # pair -> two shards, alternating by in-degree for load balance
    shard_of = np.empty(N_NODES, np.int8)
    for g in range(NW):
        nodes = np.flatnonzero(assign == g)
        so = nodes[np.argsort(-deg[nodes], kind="stable")]
        shard_of[so[0::2]] = 2 * g
        shard_of[so[1::2]] = 2 * g + 1

    # per-shard tile packing
    perm = np.empty((NC, SH), np.int64)
    row_of = np.empty(N_NODES, np.int64)
    for k in range(NC):
        nodes = np.flatnonzero(shard_of == k)
        perm[k] = nodes[_pack_tiles(cnt[nodes])]
        row_of[perm[k]] = k * SHP + np.arange(SH)
    w_fin = assign[src]                          # == row_of[src] // WSZ

    # per-(tile,window) column counts, uniform across cores
    cc = np.zeros((NC, T, P, NW), np.int64)
    for k in range(NC):
        cv = np.vstack([cnt[perm[k]], np.zeros((SHP - SH, NW), np.int64)])
        cc[k] = cv.reshape(T, P, NW)
    nw = cc.max(axis=(0, 2))                       # [T, NW] per-tile need
    nw[:, 0] = np.maximum(nw[:, 0], 1)             # window 0 covers all tiles

    # chunks per window: consecutive tiles, per-tile count padded to chunk
    # max (nbar); one dma_gather per chunk.
    wchunks = [[] for _ in range(NW)]              # (t0, t1, nbar)
    for w in range(NW):
        t0 = 0
        while t0 < T:
            t1 = t0
            nbar = 0
            while t1 < T:
                nb = max(nbar, int(nw[t1, w]))
                if (t1 - t0 + 1) * nb > CHUNK_COLS and t1 > t0:
                    break
                nbar = nb
                t1 += 1
            if nbar == 0:
                t1 = min(t0 + CHUNK_COLS, T)       # all-zero stretch
                wchunks[w].append((t0, t1, 0))
            else:
                wchunks[w].append((t0, t1, nbar))
            t0 = t1
    # realized per-tile slot width
    nbar_tw = np.zeros((T, NW), np.int64)
    for w in range(NW):
        for (t0, t1, nbar) in wchunks[w]:
            nbar_tw[t0:t1, w] = nbar
    C = int(nbar_tw.sum())

    # slot values: for each core, per (t, w): list of window-relative source
    # rows of window-w in-edges of each node, padded to nbar_tw[t,w]
    r = row_of[dst]
    k_e = r // SHP
    q = r % SHP
    t_e = q // P
    p_e = q % P
    okey = (((k_e * T + t_e) * P + p_e) * NW + w_fin)
    order = np.argsort(okey, kind="stable")
    ks, ts, ps, ws = k_e[order], t_e[order], p_e[order], w_fin[order]
    val = (row_of[src] - w_fin * WSZ)[order]
    ok_s = okey[order]
    first = np.r_[True, ok_s[1:] != ok_s[:-1]]
    starts = np.flatnonzero(first)
    gid = np.cumsum(first) - 1
    j = np.arange(len(ok_s)) - starts[gid]

    # slot array organized [NC, T, NW, nbar, P] ragged via flat offsets
    slotbase = np.concatenate([[0], np.cumsum(nbar_tw.reshape(-1))])  # T*NW+1
    slots = np.full((NC, int(slotbase[-1]), P), PADROW, np.int32)
    sidx = slotbase[ts * NW + ws] + j
    slots[ks, sidx, ps] = val

    # int16 packed idx stream per (w, chunk): flat i = (trel*nbar + c)*128+p
    # -> [p%16, i//16]; replicated over the 16-row groups of 128 partitions.
    # order segments by end tile so early tiles' accumulators complete (and
    # their per-tile chains can overlap) while later segments still gather
    raw = []
    for w in range(NW):
        for (t0, t1, nbar) in wchunks[w]:
            if nbar == 0:
                continue
            raw.append((w, t0, t1, nbar))
    raw.sort(key=lambda s: (s[2], s[1], s[0]))
    seglist = []       # (w, t0, t1, nbar, ioff, n_idx)
    ioff = 0
    for (w, t0, t1, nbar) in raw:
        n_idx = (t1 - t0) * nbar * P
        seglist.append((w, t0, t1, nbar, ioff, n_idx))
        ioff += n_idx // 16
    IDXC = ioff

    idxpack = np.empty((NC, P, IDXC), np.int16)
    for k in range(NC):
        for (w, t0, t1, nbar, off, n_idx) in seglist:
            vs = [slots[k, slotbase[t * NW + w]:slotbase[t * NW + w] + nbar]
                  for t in range(t0, t1)]
            m = np.concatenate(vs, 0).reshape(n_idx)      # (trel, c, p) flat
            blk = m.reshape(n_idx // 16, 16).T            # [16, n_idx/16]
            idxpack[k, :, off:off + n_idx // 16] = np.tile(blk, (8, 1))

    # per-(tile,partition) 1/max(deg,1)
    dinv = np.zeros((NC, P, T), np.float32)
    for k in range(NC):
        dv = (1.0 / np.maximum(deg[perm[k]], 1)).astype(np.float32)
        dv = np.pad(dv, (0, SHP - SH))
        dinv[k] = dv.reshape(T, P).T

    return dict(C=C, seglist=seglist, idxpack=idxpack, dinv=dinv, perm=perm,
                IDXC=IDXC, nbar_tw=nbar_tw)


def _build_program(meta):
    seglist, IDXC = meta["seglist"], meta["IDXC"]

    nc = bacc.Bacc("TRN2", target_bir_lowering=False, debug=False,
                   enable_asserts=False, num_devices=NC, num_swdge_queues=4)

    xtab_d = nc.dram_tensor("xtab", [NROWS, D], F32, kind="ExternalInput")
    idx_d = nc.dram_tensor("idx", [P, IDXC], I16, kind="ExternalInput")
    dinv_d = nc.dram_tensor("dinv", [P, T], F32, kind="ExternalInput")
    pmask_d = nc.dram_tensor("pmask", [P, 1], F32, kind="ExternalInput")
    w1_d = nc.dram_tensor("w1", [D, D], BF16, kind="ExternalInput")
    w2_d = nc.dram_tensor("w2", [D, D], BF16, kind="ExternalInput")
    w3_d = nc.dram_tensor("w3", [D, P], BF16, kind="ExternalInput")
    w4_d = nc.dram_tensor("w4", [P, 40], BF16, kind="ExternalInput")
    b1_d = nc.dram_tensor("b1", [D, 1], F32, kind="ExternalInput")
    b2_d = nc.dram_tensor("b2", [D, 1], F32, kind="ExternalInput")
    b3_d = nc.dram_tensor("b3", [P, 1], F32, kind="ExternalInput")
    b4_d = nc.dram_tensor("b4", [40, 1], F32, kind="ExternalInput")
    outT_d = nc.dram_tensor("outT", [40, SHP], F32, kind="ExternalOutput")

    t2self = nc.dram_tensor("t2self", [SHP, D], F32)
    t2full = nc.dram_tensor("t2full", [NROWS, D], F32)

    from contextlib import ExitStack
    with tile.TileContext(nc) as tc, ExitStack() as es:
        const = es.enter_context(tc.tile_pool(name="const", bufs=1))
        gpool = es.enter_context(tc.tile_pool(name="gpool", bufs=2))
        small = es.enter_context(tc.tile_pool(name="small", bufs=3))
        psum = es.enter_context(tc.tile_pool(name="psum", bufs=3, space="PSUM"))

        idx_s = const.tile([P, IDXC], I16)
        ic0 = seglist[0][5] // 16 if seglist else IDXC
        nc.sync.dma_start(out=idx_s[:, :ic0], in_=idx_d[:, :ic0])
        nc.sync.dma_start(out=idx_s[:, ic0:], in_=idx_d[:, ic0:])
        dinv_s = const.tile([P, T], F32)
        nc.sync.dma_start(out=dinv_s[:], in_=dinv_d[:])
        pmask_s = const.tile([P, 1], F32)
        nc.sync.dma_start(out=pmask_s[:], in_=pmask_d[:])
        w1_s = const.tile([D, D], BF16)
        nc.sync.dma_start(out=w1_s[:], in_=w1_d[:])
        w2_s = const.tile([D, D], BF16)
        nc.sync.dma_start(out=w2_s[:], in_=w2_d[:])
        w3_s = const.tile([D, P], BF16)
        nc.sync.dma_start(out=w3_s[:], in_=w3_d[:])
        w4_s = const.tile([P, 40], BF16)
        nc.sync.dma_start(out=w4_s[:], in_=w4_d[:])
        b1_s = const.tile([D, 1], F32)
        nc.sync.dma_start(out=b1_s[:], in_=b1_d[:])
        b2_s = const.tile([D, 1], F32)
        nc.sync.dma_start(out=b2_s[:], in_=b2_d[:])
        b3_s = const.tile([P, 1], F32)
        nc.sync.dma_start(out=b3_s[:], in_=b3_d[:])
        b4_s = const.tile([40, 1], F32)
        nc.sync.dma_start(out=b4_s[:], in_=b4_d[:])
        ident = const.tile([P, P], F32)
        make_identity(nc, ident[:])
        outT_s = const.tile([40, SHP], F32)
        acc = const.tile([P, T, D], BF16)

        # tiles become final after their last covering segment (in seglist
        # order); emit their per-tile chains right there so chain work
        # overlaps later segments' gathers/trees on every engine stream
        last_seg = [0] * T
        for si, (w, t0, t1, nbar, off, n_idx) in enumerate(seglist):
            for t in range(t0, t1):
                last_seg[t] = max(last_seg[t], si)
        done_after = [[] for _ in seglist]
        for t in range(T):
            done_after[last_seg[t]].append(t)

        def layer(tab_ap, w_s, b_s, last):
            def chain(t, aggs_g, a):
                pt = psum.tile([D, P], F32, tag="tp", space="PSUM")
                nc.tensor.transpose(pt[:], aggs_g[:, t - a, :], ident[:])
                rhs = small.tile([D, P], BF16, tag="rhs")
                nc.scalar.activation(
                    rhs[:], pt[:], mybir.ActivationFunctionType.Copy)
                pm = psum.tile([D, P], F32, tag="mm", space="PSUM")
                nc.tensor.matmul(pm[:], lhsT=w_s[:], rhs=rhs[:],
                                 start=True, stop=True)
                tT = small.tile([D, P], BF16 if last else F32, tag="tT")
                nc.scalar.activation(
                    tT[:], pm[:], mybir.ActivationFunctionType.Sigmoid,
                    bias=b_s[:, :1])
                if not last:
                    pb = psum.tile([P, D], F32, tag="tp", space="PSUM")
                    nc.tensor.transpose(pb[:], tT[:], ident[:D, :D])
                    t2t = small.tile([P, D], F32, tag="t2t")
                    # Act copy; the T-1 tile zeroes the 44 shard-pad rows via
                    # the pmask scale so every window keeps a zero PAD target
                    nc.scalar.activation(
                        t2t[:], pb[:], mybir.ActivationFunctionType.Copy,
                        scale=pmask_s[:, :1] if t == T - 1 else 1.0)
                    nc.sync.dma_start(
                        out=t2self[t * P:(t + 1) * P, :], in_=t2t[:])
                else:
                    p3 = psum.tile([P, P], F32, tag="mm", space="PSUM")
                    nc.tensor.matmul(p3[:], lhsT=w3_s[:], rhs=tT[:],
                                     start=True, stop=True)
                    h3 = small.tile([P, P], BF16, tag="h3")
                    nc.scalar.activation(
                        h3[:], p3[:], mybir.ActivationFunctionType.Relu,
                        bias=b3_s[:, :1])
                    p4 = psum.tile([40, P], F32, tag="mm", space="PSUM")
                    nc.tensor.matmul(p4[:], lhsT=w4_s[:], rhs=h3[:],
                                     start=True, stop=True)
                    nc.scalar.activation(
                        outT_s[:, t * P:(t + 1) * P], p4[:],
                        mybir.ActivationFunctionType.Identity,
                        bias=b4_s[:, :1])

            nc.vector.memset(acc[:], 0.0)
            for si, (w, t0, t1, nbar, off, n_idx) in enumerate(seglist):
                ntile = t1 - t0
                G = gpool.tile([P, ntile, nbar, D], F32, tag="G")
                Gfull = G[:]
                G3 = bass.AP(tensor=Gfull.tensor, offset=Gfull.offset,
                             ap=[list(Gfull.ap[0]), [D, ntile * nbar], [1, D]])
                assert n_idx <= 8192, (
                    f"gather of {n_idx} idxs exceeds the 8192-descriptor "
                    f"SWDGE limit (crashes the device)")
                nc.gpsimd.dma_gather(
                    out_ap=G3,
                    in_ap=tab_ap[w * WSZ:(w + 1) * WSZ],
                    idxs_ap=idx_s[:, off:off + n_idx // 16],
                    num_idxs=n_idx, num_idxs_reg=n_idx,
                    elem_size=D, queue_num=si % 4, single_packet=False)
                # tree-reduce axis c: level 0 folds f32 pairs into a
                # contiguous bf16 tile (emulated precision cost: +4e-4 rel),
                # later levels run in bf16 at 2x DVE rate
                if nbar == 1:
                    radd = Gfull[:, :, 0, :]
                else:
                    n = nbar
                    if n % 2:
                        nc.vector.tensor_tensor(
                            out=Gfull[:, :, 0, :], in0=Gfull[:, :, 0, :],
                            in1=Gfull[:, :, n - 1, :], op=mybir.AluOpType.add)
                    h = n // 2
                    Gb = gpool.tile([P, ntile, h, D], BF16, tag="Gb")
                    nc.vector.tensor_tensor(
                        out=Gb[:], in0=Gfull[:, :, :h, :],
                        in1=Gfull[:, :, h:2 * h, :], op=mybir.AluOpType.add)
                    n = h
                    while n > 1:
                        h = n // 2
                        if n % 2:
                            nc.vector.tensor_tensor(
                                out=Gb[:, :, 0, :], in0=Gb[:, :, 0, :],
                                in1=Gb[:, :, n - 1, :], op=mybir.AluOpType.add)
                        nc.vector.tensor_tensor(
                            out=Gb[:, :, :h, :], in0=Gb[:, :, :h, :],
                            in1=Gb[:, :, h:2 * h, :], op=mybir.AluOpType.add)
                        n = h
                    radd = Gb[:, :, 0, :]
                nc.vector.tensor_tensor(
                    out=acc[:, t0:t1, :], in0=acc[:, t0:t1, :],
                    in1=radd, op=mybir.AluOpType.add)
                grp = done_after[si]
                if grp:
                    a, b = grp[0], grp[-1] + 1
                    aggs_g = small.tile([P, b - a, D], F32, tag="aggs")
                    dv = dinv_s[:, a:b]
                    dvb = bass.AP(tensor=dv.tensor, offset=dv.offset,
                                  ap=[list(dv.ap[0]), list(dv.ap[1]), [0, D]])
                    nc.vector.tensor_tensor(
                        out=aggs_g[:], in0=acc[:, a:b, :], in1=dvb,
                        op=mybir.AluOpType.mult)
                    for t in grp:
                        chain(t, aggs_g, a)

        layer(xtab_d[:], w1_s, b1_s, last=False)
        nc.gpsimd.collective_compute(
            "AllGather",
            mybir.AluOpType.bypass,
            replica_groups=[list(range(NC))],
            ins=[t2self.ap().opt()],
            outs=[t2full.ap().opt()],
        )
        layer(t2full[:], w2_s, b2_s, last=True)
        nc.sync.dma_start(out=outT_d[:], in_=outT_s[:])

    nc.compile()
    return nc


def kernel(features, edge_index, W1, b1, W2, b2, W3, b3, W4, b4):
    meta = _preprocess(edge_index)
    perm = meta["perm"]

    nc = _build_program(meta)

    xtab = np.zeros((NROWS, D), np.float32)
    X = np.asarray(features, np.float32)
    for k in range(NC):
        xtab[k * SHP:k * SHP + SH] = X[perm[k]]

    pmask = np.zeros((P, 1), np.float32)
    pmask[:SH - (T - 1) * P] = 1.0
    common = dict(
        xtab=xtab,
        pmask=pmask,
        w1=np.asarray(W1, np.float32).astype(np.float32),
        w2=np.asarray(W2, np.float32).astype(np.float32),
        w3=np.asarray(W3, np.float32).astype(np.float32),
        w4=np.asarray(W4, np.float32).astype(np.float32),
        b1=np.asarray(b1, np.float32).reshape(D, 1),
        b2=np.asarray(b2, np.float32).reshape(D, 1),
        b3=np.asarray(b3, np.float32).reshape(P, 1),
        b4=np.asarray(b4, np.float32).reshape(40, 1),
    )
    # weights are declared bf16 on device; ml_dtypes cast here
    import ml_dtypes
    for wn in ("w1", "w2", "w3", "w4"):
        common[wn] = common[wn].astype(ml_dtypes.bfloat16)

    in_maps = [dict(common, idx=meta["idxpack"][k],
                    dinv=meta["dinv"][k]) for k in range(NC)]

    results = _run_spmd_timed(nc, in_maps,
                              reps=int(os.environ.get("KERNEL_REPS", "0")))

    out = np.empty((N_NODES, 40), np.float32)
    for k in range(NC):
        outT = np.asarray(results[k]["outT"], np.float32)
        out[perm[k]] = outT[:, :SH].T
    return out


def _run_spmd_timed(nc, in_maps, reps=0):
    """Device_put inputs once; repeated warm timed executions (NTFF profiling
    is unavailable under this axon client, so warm wall-clock is the
    metric)."""
    import time
    import jax
    from jax.sharding import Mesh, PartitionSpec
    from jax.experimental.shard_map import shard_map
    from concourse import bass2jax, mybir as mb

    bass2jax.install_neuronx_cc_hook()
    n_cores = len(in_maps)
    partition_name = (nc.partition_id_tensor.name
                      if nc.partition_id_tensor else None)
    in_names, out_names, out_avals, zero_outs = [], [], [], []
    for alloc in nc.m.functions[0].allocations:
        if not isinstance(alloc, mb.MemoryLocationSet):
            continue
        name = alloc.memorylocations[0].name
        if alloc.kind == "ExternalInput":
            if name != partition_name:
                in_names.append(name)
        elif alloc.kind == "ExternalOutput":
            shape = tuple(alloc.tensor_shape)
            dtype = mb.dt.np(alloc.dtype)
            out_avals.append(jax.core.ShapedArray(shape, dtype))
            zero_outs.append(np.zeros(shape, dtype))
            out_names.append(name)
    n_params = len(in_names)
    n_outs = len(out_avals)
    all_in_names = list(in_names) + list(out_names)
    if partition_name is not None:
        all_in_names.append(partition_name)

    def _body(*args):
        operands = list(args)
        if partition_name is not None:
            operands.append(bass2jax.partition_id_tensor())
        return tuple(bass2jax._bass_exec_p.bind(
            *operands, out_avals=tuple(out_avals),
            in_names=tuple(all_in_names), out_names=tuple(out_names),
            lowering_input_output_aliases=(),
            sim_require_finite=True, sim_require_nnan=True, nc=nc))

    devices = jax.devices()[:n_cores]
    mesh = Mesh(np.asarray(devices), ("core",))
    sharded = jax.jit(
        shard_map(_body, mesh=mesh,
                  in_specs=(PartitionSpec("core"),) * (n_params + n_outs),
                  out_specs=(PartitionSpec("core"),) * n_outs,
                  check_rep=False),
        keep_unused=True)

    concat_in = [np.concatenate([np.asarray(m[name]) for m in in_maps], axis=0)
                 for name in in_names]
    dev_in = [jax.device_put(a) for a in concat_in]
    jax.block_until_ready(dev_in)

    dev_zeros = [jax.device_put(np.zeros((n_cores * z.shape[0],
                                          *z.shape[1:]), z.dtype))
                 for z in zero_outs]
    jax.block_until_ready(dev_zeros)

    def one_call():
        t0 = time.perf_counter()
        outs = sharded(*dev_in, *dev_zeros)
        jax.block_until_ready(outs)
        return time.perf_counter() - t0, outs

    _, outs = one_call()            # compile + first exec
    if reps > 0:
        times = [one_call()[0] for _ in range(reps)]
        best = min(times)
        print(f"HW exec time: {best * 1e9:.0f} ns")
        print("wall times (s):", [f"{t:.4f}" for t in times])
    return [
        {name: np.asarray(outs[i]).reshape(n_cores, *out_avals[i].shape)[c]
         for i, name in enumerate(out_names)}
        for c in range(n_cores)
    ]


if __name__ == "__main__":
    d = np.load("/tmp/inputs.npz")
    out = kernel(**{k: d[k] for k in d.files})
    ref = np.load("/tmp/ref.npy")
    err = np.abs(out - ref).max() / np.abs(ref).max()
    print("Relative error:", err)


# revision 7
# speedup vs baseline: 1.0519x; 1.0151x over previous
"""HGCN (2x hyperbolic GCN layer + MLP head) as a distributed Bass/Tile kernel
for 8 trn2 NeuronCores — v2: SWDGE dma_gather edge gather.

Math (as baseline): logmap0(expmap0(v)) == v for this data, so
    t2  = sigmoid(meanagg(X) @ W1 + b1)
    t3  = sigmoid(meanagg(t2) @ W2 + b2)
    out = relu(t3 @ W3 + b3) @ W4 + b4

Distribution: destination nodes sharded 8 ways (12500/core + 44 pad = 12544
= T*128 rows, T=98 tiles).  Sources are gathered from a replicated f32 table
(25.6MB; layer 2's table arrives via AllGather).  dma_gather indices are
int16, so the table is split into 4 windows of 25088 rows; shard 2w's zero
pad rows sit at relative row 12500 inside window w, giving every window a
zero PAD target.

Slot grid per core: for window w, tile t gets nw[t][w] slot columns; tiles
are grouped into chunks with a uniform per-tile column count (padded to the
chunk max), so one dma_gather fills [128, ntile*nbar, 64] and the tree
reduce runs as log2(nbar) big strided DVE adds, accumulated per tile into a
persistent ACC [128, T, 64].  Nodes are packed into tiles by a greedy vector
bin-packing that minimizes the per-window column maxima.
"""

import os
import numpy as np

import concourse.bass as bass
import concourse.bacc as bacc
import concourse.tile as tile
from concourse import mybir
from concourse.masks import make_identity

NC = 8
P = 128
D = 64
NW = 4
CHUNK_COLS = 28    # max slot columns per gather chunk (28*128 = 3584 idxs,
                   # well under the 8192-descriptor multi-packet SWDGE cap;
                   # smaller chunks pad less, and Pool has dispatch slack)

BF16 = mybir.dt.bfloat16
F32 = mybir.dt.float32
I16 = mybir.dt.int16

N_NODES = 100000
SH = N_NODES // NC            # 12500
T = (SH + P - 1) // P         # 98
SHP = T * P                   # 12544
NROWS = NC * SHP              # 100352
WSZ = NROWS // NW             # 25088
PADROW = SH                   # relative zero row inside every window


def _pack_tiles(cntk):
    """Order shard nodes (rows of cntk [n, NW]) into tiles of 128: greedy
    vector packing inside 16-tile blocks (minimizes per-tile window maxima),
    tiles within a block reordered by decreasing max so adaptive chunks pad
    little."""
    n = len(cntk)
    osort = np.argsort(-(cntk.max(1) * 1000 + cntk.sum(1)), kind="stable")
    B = 16 * P
    out = []
    for s0 in range(0, n, B):
        blk = osort[s0:s0 + B]
        ntile = (len(blk) + P - 1) // P
        maxs = np.zeros((ntile, NW), np.int64)
        fills = np.zeros(ntile, np.int64)
        asn = [[] for _ in range(ntile)]
        for nd in blk:
            cv = cntk[nd]
            inc = (np.maximum(maxs, cv) - maxs).sum(1)
            inc[fills >= P] = 1 << 30
            ti = int(np.argmin(inc))
            asn[ti].append(nd)
            fills[ti] += 1
            np.maximum(maxs[ti], cv, out=maxs[ti])
        key = [-(maxs[ti].max() * 1000 + maxs[ti].sum()) for ti in range(ntile)]
        for ti in np.argsort(key, kind="stable"):
            out.extend(asn[ti])
    return np.asarray(out, np.int64)


def _preprocess(edge_index):
    """Host-side layout preprocessing (index shuffling only)."""
    src = np.asarray(edge_index[0], np.int64)
    dst = np.asarray(edge_index[1], np.int64)
    deg = np.bincount(dst, minlength=N_NODES).astype(np.int64)
    deg_out = np.bincount(src, minlength=N_NODES).astype(np.int64)

    # --- window assignment: choose each node's shard-PAIR (= its gather
    # window when it appears as a source) to balance every destination's
    # in-neighborhood across the 4 windows.
    order = np.argsort(src, kind="stable")
    dst_by_src = dst[order]
    ptr = np.zeros(N_NODES + 1, np.int64)
    np.cumsum(np.bincount(src, minlength=N_NODES), out=ptr[1:])
    targ = np.ceil(deg / NW).astype(np.int64)
    cnt = np.zeros((N_NODES, NW), np.int32)     # per (dst, window) counts
    cap = 2 * SH                                # 25000 nodes per pair
    sizes = np.zeros(NW, np.int64)
    assign = np.full(N_NODES, -1, np.int64)
    for s in np.argsort(-deg_out, kind="stable"):
        nbrs = dst_by_src[ptr[s]:ptr[s + 1]]
        if len(nbrs):
            over = (cnt[nbrs] >= targ[nbrs][:, None]).sum(0)
        else:
            over = np.zeros(NW, np.int64)
        g = int(np.argmin(over + (sizes >= cap) * 10 ** 9
                          + 0.001 * (sizes / cap)))
        assign[s] = g
        sizes[g] += 1
        if len(nbrs):
            cnt[nbrs, g] += 1
    # the += above undercounts duplicate edges; recompute exact counts
    cnt = np.zeros((N_NODES, NW), np.int64)
    np.add.at(cnt, (dst, assign[src]), 1)

    # pair -> two shards, alternating by in-degree for load balance
    shard_of = np.empty(N_NODES, np.int8)
    for g in range(NW):
        nodes = np.flatnonzero(assign == g)
        so = nodes[np.argsort(-deg[nodes], kind="stable")]
        shard_of[so[0::2]] = 2 * g
        shard_of[so[1::2]] = 2 * g + 1

    # per-shard tile packing
    perm = np.empty((NC, SH), np.int64)
    row_of = np.empty(N_NODES, np.int64)
    for k in range(NC):
        nodes = np.flatnonzero(shard_of == k)
        perm[k] = nodes[_pack_tiles(cnt[nodes])]
        row_of[perm[k]] = k * SHP + np.arange(SH)
    w_fin = assign[src]                          # == row_of[src] // WSZ

    # per-(tile,window) column counts, uniform across cores
    cc = np.zeros((NC, T, P, NW), np.int64)
    for k in range(NC):
        cv = np.vstack([cnt[perm[k]], np.zeros((SHP - SH, NW), np.int64)])
        cc[k] = cv.reshape(T, P, NW)
    nw = cc.max(axis=(0, 2))                       # [T, NW] per-tile need
    nw[:, 0] = np.maximum(nw[:, 0], 1)             # window 0 covers all tiles

    # chunks per window: consecutive tiles, per-tile count padded to chunk
    # max (nbar); one dma_gather per chunk.
    wchunks = [[] for _ in range(NW)]              # (t0, t1, nbar)
    for w in range(NW):
        t0 = 0
        while t0 < T:
            t1 = t0
            nbar = 0
            while t1 < T:
                nb = max(nbar, int(nw[t1, w]))
                if (t1 - t0 + 1) * nb > CHUNK_COLS and t1 > t0:
                    break
                nbar = nb
                t1 += 1
            if nbar == 0:
                t1 = min(t0 + CHUNK_COLS, T)       # all-zero stretch
                wchunks[w].append((t0, t1, 0))
            else:
                wchunks[w].append((t0, t1, nbar))
            t0 = t1
    # realized per-tile slot width
    nbar_tw = np.zeros((T, NW), np.int64)
    for w in range(NW):
        for (t0, t1, nbar) in wchunks[w]:
            nbar_tw[t0:t1, w] = nbar
    C = int(nbar_tw.sum())

    # slot values: for each core, per (t, w): list of window-relative source
    # rows of window-w in-edges of each node, padded to nbar_tw[t,w]
    r = row_of[dst]
    k_e = r // SHP
    q = r % SHP
    t_e = q // P
    p_e = q % P
    okey = (((k_e * T + t_e) * P + p_e) * NW + w_fin)
    order = np.argsort(okey, kind="stable")
    ks, ts, ps, ws = k_e[order], t_e[order], p_e[order], w_fin[order]
    val = (row_of[src] - w_fin * WSZ)[order]
    ok_s = okey[order]
    first = np.r_[True, ok_s[1:] != ok_s[:-1]]
    starts = np.flatnonzero(first)
    gid = np.cumsum(first) - 1
    j = np.arange(len(ok_s)) - starts[gid]

    # slot array organized [NC, T, NW, nbar, P] ragged via flat offsets
    slotbase = np.concatenate([[0], np.cumsum(nbar_tw.reshape(-1))])  # T*NW+1
    slots = np.full((NC, int(slotbase[-1]), P), PADROW, np.int32)
    sidx = slotbase[ts * NW + ws] + j
    slots[ks, sidx, ps] = val

    # int16 packed idx stream per (w, chunk): flat i = (trel*nbar + c)*128+p
    # -> [p%16, i//16]; replicated over the 16-row groups of 128 partitions.
    # order segments by end tile so early tiles' accumulators complete (and
    # their per-tile chains can overlap) while later segments still gather
    raw = []
    for w in range(NW):
        for (t0, t1, nbar) in wchunks[w]:
            if nbar == 0:
                continue
            raw.append((w, t0, t1, nbar))
    raw.sort(key=lambda s: (s[2], s[1], s[0]))
    seglist = []       # (w, t0, t1, nbar, ioff, n_idx)
    ioff = 0
    for (w, t0, t1, nbar) in raw:
        n_idx = (t1 - t0) * nbar * P
        seglist.append((w, t0, t1, nbar, ioff, n_idx))
        ioff += n_idx // 16
    IDXC = ioff

    idxpack = np.empty((NC, P, IDXC), np.int16)
    for k in range(NC):
        for (w, t0, t1, nbar, off, n_idx) in seglist:
            vs = [slots[k, slotbase[t * NW + w]:slotbase[t * NW + w] + nbar]
                  for t in range(t0, t1)]
            m = np.concatenate(vs, 0).reshape(n_idx)      # (trel, c, p) flat
            blk = m.reshape(n_idx // 16, 16).T            # [16, n_idx/16]
            idxpack[k, :, off:off + n_idx // 16] = np.tile(blk, (8, 1))

    # per-(tile,partition) 1/max(deg,1)
    dinv = np.zeros((NC, P, T), np.float32)
    for k in range(NC):
        dv = (1.0 / np.maximum(deg[perm[k]], 1)).astype(np.float32)
        dv = np.pad(dv, (0, SHP - SH))
        dinv[k] = dv.reshape(T, P).T

    return dict(C=C, seglist=seglist, idxpack=idxpack, dinv=dinv, perm=perm,
                IDXC=IDXC, nbar_tw=nbar_tw)


def _build_program(meta):
    seglist, IDXC = meta["seglist"], meta["IDXC"]

    nc = bacc.Bacc("TRN2", target_bir_lowering=False, debug=False,
                   enable_asserts=False, num_devices=NC, num_swdge_queues=4)

    xtab_d = nc.dram_tensor("xtab", [NROWS, D], F32, kind="ExternalInput")
    idx_d = nc.dram_tensor("idx", [P, IDXC], I16, kind="ExternalInput")
    dinv_d = nc.dram_tensor("dinv", [P, T], F32, kind="ExternalInput")
    pmask_d = nc.dram_tensor("pmask", [P, 1], F32, kind="ExternalInput")
    w1_d = nc.dram_tensor("w1", [D, D], BF16, kind="ExternalInput")
    w2_d = nc.dram_tensor("w2", [D, D], BF16, kind="ExternalInput")
    w3_d = nc.dram_tensor("w3", [D, P], BF16, kind="ExternalInput")
    w4_d = nc.dram_tensor("w4", [P, 40], BF16, kind="ExternalInput")
    b1_d = nc.dram_tensor("b1", [D, 1], F32, kind="ExternalInput")
    b2_d = nc.dram_tensor("b2", [D, 1], F32, kind="ExternalInput")
    b3_d = nc.dram_tensor("b3", [P, 1], F32, kind="ExternalInput")
    b4_d = nc.dram_tensor("b4", [40, 1], F32, kind="ExternalInput")
    outT_d = nc.dram_tensor("outT", [40, SHP], F32, kind="ExternalOutput")

    t2self = nc.dram_tensor("t2self", [SHP, D], F32)
    t2full = nc.dram_tensor("t2full", [NROWS, D], F32)

    from contextlib import ExitStack
    with tile.TileContext(nc) as tc, ExitStack() as es:
        const = es.enter_context(tc.tile_pool(name="const", bufs=1))
        gpool = es.enter_context(tc.tile_pool(name="gpool", bufs=2))
        small = es.enter_context(tc.tile_pool(name="small", bufs=3))
        psum = es.enter_context(tc.tile_pool(name="psum", bufs=3, space="PSUM"))

        idx_s = const.tile([P, IDXC], I16)
        ic0 = seglist[0][5] // 16 if seglist else IDXC
        nc.sync.dma_start(out=idx_s[:, :ic0], in_=idx_d[:, :ic0])
        nc.sync.dma_start(out=idx_s[:, ic0:], in_=idx_d[:, ic0:])
        dinv_s = const.tile([P, T], F32)
        nc.sync.dma_start(out=dinv_s[:], in_=dinv_d[:])
        pmask_s = const.tile([P, 1], F32)
        nc.sync.dma_start(out=pmask_s[:], in_=pmask_d[:])
        w1_s = const.tile([D, D], BF16)
        nc.sync.dma_start(out=w1_s[:], in_=w1_d[:])
        w2_s = const.tile([D, D], BF16)
        nc.sync.dma_start(out=w2_s[:], in_=w2_d[:])
        w3_s = const.tile([D, P], BF16)
        nc.sync.dma_start(out=w3_s[:], in_=w3_d[:])
        w4_s = const.tile([P, 40], BF16)
        nc.sync.dma_start(out=w4_s[:], in_=w4_d[:])
        b1_s = const.tile([D, 1], F32)
        nc.sync.dma_start(out=b1_s[:], in_=b1_d[:])
        b2_s = const.tile([D, 1], F32)
        nc.sync.dma_start(out=b2_s[:], in_=b2_d[:])
        b3_s = const.tile([P, 1], F32)
        nc.sync.dma_start(out=b3_s[:], in_=b3_d[:])
        b4_s = const.tile([40, 1], F32)
        nc.sync.dma_start(out=b4_s[:], in_=b4_d[:])
        ident = const.tile([P, P], F32)
        make_identity(nc, ident[:])
        outT_s = const.tile([40, SHP], F32)
        acc = const.tile([P, T, D], BF16)

        # tiles become final after their last covering segment (in seglist
        # order); emit their per-tile chains right there so chain work
        # overlaps later segments' gathers/trees on every engine stream
        last_seg = [0] * T
        for si, (w, t0, t1, nbar, off, n_idx) in enumerate(seglist):
            for t in range(t0, t1):
                last_seg[t] = max(last_seg[t], si)
        done_after = [[] for _ in seglist]
        for t in range(T):
            done_after[last_seg[t]].append(t)

        def layer(tab_ap, w_s, b_s, last):
            def chain(t, aggs_g, a):
                pt = psum.tile([D, P], F32, tag="tp", space="PSUM")
                nc.tensor.transpose(pt[:], aggs_g[:, t - a, :], ident[:])
                rhs = small.tile([D, P], BF16, tag="rhs")
                nc.scalar.activation(
                    rhs[:], pt[:], mybir.ActivationFunctionType.Copy)
                pm = psum.tile([D, P], F32, tag="mm", space="PSUM")
                nc.tensor.matmul(pm[:], lhsT=w_s[:], rhs=rhs[:],
                                 start=True, stop=True)
                tT = small.tile([D, P], BF16 if last else F32, tag="tT")
                nc.scalar.activation(
                    tT[:], pm[:], mybir.ActivationFunctionType.Sigmoid,
                    bias=b_s[:, :1])
                if not last:
                    pb = psum.tile([P, D], F32, tag="tp", space="PSUM")
                    nc.tensor.transpose(pb[:], tT[:], ident[:D, :D])
                    t2t = small.tile([P, D], F32, tag="t2t")
                    # Act copy; the T-1 tile zeroes the 44 shard-pad rows via
                    # the pmask scale so every window keeps a zero PAD target
                    nc.scalar.activation(
                        t2t[:], pb[:], mybir.ActivationFunctionType.Copy,
                        scale=pmask_s[:, :1] if t == T - 1 else 1.0)
                    nc.sync.dma_start(
                        out=t2self[t * P:(t + 1) * P, :], in_=t2t[:])
                else:
                    p3 = psum.tile([P, P], F32, tag="mm", space="PSUM")
                    nc.tensor.matmul(p3[:], lhsT=w3_s[:], rhs=tT[:],
                                     start=True, stop=True)
                    h3 = small.tile([P, P], BF16, tag="h3")
                    nc.scalar.activation(
                        h3[:], p3[:], mybir.ActivationFunctionType.Relu,
                        bias=b3_s[:, :1])
                    p4 = psum.tile([40, P], F32, tag="mm", space="PSUM")
                    nc.tensor.matmul(p4[:], lhsT=w4_s[:], rhs=h3[:],
                                     start=True, stop=True)
                    nc.scalar.activation(
                        outT_s[:, t * P:(t + 1) * P], p4[:],
                        mybir.ActivationFunctionType.Identity,
                        bias=b4_s[:, :1])

            nc.vector.memset(acc[:], 0.0)
            for si, (w, t0, t1, nbar, off, n_idx) in enumerate(seglist):
                ntile = t1 - t0
                G = gpool.tile([P, ntile, nbar, D], F32, tag="G")
                Gfull = G[:]
                G3 = bass.AP(tensor=Gfull.tensor, offset=Gfull.offset,
                             ap=[list(Gfull.ap[0]), [D, ntile * nbar], [1, D]])
                assert n_idx <= 8192, (
                    f"gather of {n_idx} idxs exceeds the 8192-descriptor "
                    f"SWDGE limit (crashes the device)")
                nc.gpsimd.dma_gather(
                    out_ap=G3,
                    in_ap=tab_ap[w * WSZ:(w + 1) * WSZ],
                    idxs_ap=idx_s[:, off:off + n_idx // 16],
                    num_idxs=n_idx, num_idxs_reg=n_idx,
                    elem_size=D, queue_num=si % 4, single_packet=False)
                # tree-reduce axis c: level 0 folds f32 pairs into a
                # contiguous bf16 tile (emulated precision cost: +4e-4 rel),
                # later levels run in bf16 at 2x DVE rate
                if nbar == 1:
                    radd = Gfull[:, :, 0, :]
                else:
                    n = nbar
                    if n % 2:
                        nc.vector.tensor_tensor(
                            out=Gfull[:, :, 0, :], in0=Gfull[:, :, 0, :],
                            in1=Gfull[:, :, n - 1, :], op=mybir.AluOpType.add)
                    h = n // 2
                    Gb = gpool.tile([P, ntile, h, D], BF16, tag="Gb")
                    nc.vector.tensor_tensor(
                        out=Gb[:], in0=Gfull[:, :, :h, :],
                        in1=Gfull[:, :, h:2 * h, :], op=mybir.AluOpType.add)
                    n = h
                    while n > 1:
                        h = n // 2
                        if n % 2:
                            nc.vector.tensor_tensor(
                                out=Gb[:, :, 0, :], in0=Gb[:, :, 0, :],
                                in1=Gb[:, :, n - 1, :], op=mybir.AluOpType.add)
                        nc.vector.tensor_tensor(
                            out=Gb[:, :, :h, :], in0=Gb[:, :, :h, :],
                            in1=Gb[:, :, h:2 * h, :], op=mybir.AluOpType.add)
                        n = h
                    radd = Gb[:, :, 0, :]
                nc.vector.tensor_tensor(
                    out=acc[:, t0:t1, :], in0=acc[:, t0:t1, :],
                    in1=radd, op=mybir.AluOpType.add)
                grp = done_after[si]
                if grp:
                    a, b = grp[0], grp[-1] + 1
                    aggs_g = small.tile([P, b - a, D], F32, tag="aggs")
                    dv = dinv_s[:, a:b]
                    dvb = bass.AP(tensor=dv.tensor, offset=dv.offset,
                                  ap=[list(dv.ap[0]), list(dv.ap[1]), [0, D]])
                    nc.vector.tensor_tensor(
                        out=aggs_g[:], in0=acc[:, a:b, :], in1=dvb,
                        op=mybir.AluOpType.mult)
                    for t in grp:
                        chain(t, aggs_g, a)
                    if last:
                        nc.sync.dma_start(
                            out=outT_d[:, a * P:b * P],
                            in_=outT_s[:, a * P:b * P])

        layer(xtab_d[:], w1_s, b1_s, last=False)
        nc.gpsimd.collective_compute(
            "AllGather",
            mybir.AluOpType.bypass,
            replica_groups=[list(range(NC))],
            ins=[t2self.ap().opt()],
            outs=[t2full.ap().opt()],
        )
        layer(t2full[:], w2_s, b2_s, last=True)

    nc.compile()
    return nc


def kernel(features, edge_index, W1, b1, W2, b2, W3, b3, W4, b4):
    meta = _preprocess(edge_index)
    perm = meta["perm"]

    nc = _build_program(meta)

    xtab = np.zeros((NROWS, D), np.float32)
    X = np.asarray(features, np.float32)
    for k in range(NC):
        xtab[k * SHP:k * SHP + SH] = X[perm[k]]

    pmask = np.zeros((P, 1), np.float32)
    pmask[:SH - (T - 1) * P] = 1.0
    common = dict(
        xtab=xtab,
        pmask=pmask,
        w1=np.asarray(W1, np.float32).astype(np.float32),
        w2=np.asarray(W2, np.float32).astype(np.float32),
        w3=np.asarray(W3, np.float32).astype(np.float32),
        w4=np.asarray(W4, np.float32).astype(np.float32),
        b1=np.asarray(b1, np.float32).reshape(D, 1),
        b2=np.asarray(b2, np.float32).reshape(D, 1),
        b3=np.asarray(b3, np.float32).reshape(P, 1),
        b4=np.asarray(b4, np.float32).reshape(40, 1),
    )
    # weights are declared bf16 on device; ml_dtypes cast here
    import ml_dtypes
    for wn in ("w1", "w2", "w3", "w4"):
        common[wn] = common[wn].astype(ml_dtypes.bfloat16)

    in_maps = [dict(common, idx=meta["idxpack"][k],
                    dinv=meta["dinv"][k]) for k in range(NC)]

    results = _run_spmd_timed(nc, in_maps,
                              reps=int(os.environ.get("KERNEL_REPS", "0")))

    out = np.empty((N_NODES, 40), np.float32)
    for k in range(NC):
        outT = np.asarray(results[k]["outT"], np.float32)
        out[perm[k]] = outT[:, :SH].T
    return out


def _run_spmd_timed(nc, in_maps, reps=0):
    """Device_put inputs once; repeated warm timed executions (NTFF profiling
    is unavailable under this axon client, so warm wall-clock is the
    metric)."""
    import time
    import jax
    from jax.sharding import Mesh, PartitionSpec
    from jax.experimental.shard_map import shard_map
    from concourse import bass2jax, mybir as mb

    bass2jax.install_neuronx_cc_hook()
    n_cores = len(in_maps)
    partition_name = (nc.partition_id_tensor.name
                      if nc.partition_id_tensor else None)
    in_names, out_names, out_avals, zero_outs = [], [], [], []
    for alloc in nc.m.functions[0].allocations:
        if not isinstance(alloc, mb.MemoryLocationSet):
            continue
        name = alloc.memorylocations[0].name
        if alloc.kind == "ExternalInput":
            if name != partition_name:
                in_names.append(name)
        elif alloc.kind == "ExternalOutput":
            shape = tuple(alloc.tensor_shape)
            dtype = mb.dt.np(alloc.dtype)
            out_avals.append(jax.core.ShapedArray(shape, dtype))
            zero_outs.append(np.zeros(shape, dtype))
            out_names.append(name)
    n_params = len(in_names)
    n_outs = len(out_avals)
    all_in_names = list(in_names) + list(out_names)
    if partition_name is not None:
        all_in_names.append(partition_name)

    def _body(*args):
        operands = list(args)
        if partition_name is not None:
            operands.append(bass2jax.partition_id_tensor())
        return tuple(bass2jax._bass_exec_p.bind(
            *operands, out_avals=tuple(out_avals),
            in_names=tuple(all_in_names), out_names=tuple(out_names),
            lowering_input_output_aliases=(),
            sim_require_finite=True, sim_require_nnan=True, nc=nc))

    devices = jax.devices()[:n_cores]
    mesh = Mesh(np.asarray(devices), ("core",))
    sharded = jax.jit(
        shard_map(_body, mesh=mesh,
                  in_specs=(PartitionSpec("core"),) * (n_params + n_outs),
                  out_specs=(PartitionSpec("core"),) * n_outs,
                  check_rep=False),
        keep_unused=True)

    concat_in = [np.concatenate([np.asarray(m[name]) for m in in_maps], axis=0)
                 for name in in_names]
    dev_in = [jax.device_put(a) for a in concat_in]
    jax.block_until_ready(dev_in)

    dev_zeros = [jax.device_put(np.zeros((n_cores * z.shape[0],
                                          *z.shape[1:]), z.dtype))
                 for z in zero_outs]
    jax.block_until_ready(dev_zeros)

    def one_call():
        t0 = time.perf_counter()
        outs = sharded(*dev_in, *dev_zeros)
        jax.block_until_ready(outs)
        return time.perf_counter() - t0, outs

    _, outs = one_call()            # compile + first exec
    if reps > 0:
        times = [one_call()[0] for _ in range(reps)]
        best = min(times)
        print(f"HW exec time: {best * 1e9:.0f} ns")
        print("wall times (s):", [f"{t:.4f}" for t in times])
    return [
        {name: np.asarray(outs[i]).reshape(n_cores, *out_avals[i].shape)[c]
         for i, name in enumerate(out_names)}
        for c in range(n_cores)
    ]


if __name__ == "__main__":
    d = np.load("/tmp/inputs.npz")
    out = kernel(**{k: d[k] for k in d.files})
    ref = np.load("/tmp/ref.npy")
    err = np.abs(out - ref).max() / np.abs(ref).max()
    print("Relative error:", err)
